# revision 16
# baseline (speedup 1.0000x reference)
"""GAU (gated attention unit, relu^2 linear attention) Trainium2 kernel.

Sharding: 8 cores = batch (4) x T-half (2).  Each core handles 2048 rows
of both the query and key/value streams of one batch.  The kv/k_sum
reduction over keys is completed with two 2-rank AllReduces (bf16
payload), each issued as soon as its half of the data is ready so both
hide under the q-feature/gate pass.

Matmuls run in bf16 with fp32 PSUM accumulation; inputs are cast to
bf16 on the host.  Bias adds ride vector/scalar ops instead of PE
matmuls: bk via a vector add, bv via a rank-1 k_sum x bv fix-up fused
into the kv unpack, and the key mask is folded into the v-projection
copy (km^2 == km for 0/1 masks) and the k_sum ones-vector.
"""
import sys

sys.path.insert(0, "/opt/trn_rl_repo")

import ml_dtypes
import numpy as np
import concourse.bass as bass
import concourse.mybir as mybir
import concourse.tile as tile
from concourse.bass_utils import run_bass_kernel_spmd

AF = mybir.ActivationFunctionType
ALU = mybir.AluOpType
F32 = mybir.dt.float32
BF16 = mybir.dt.bfloat16
NPBF = ml_dtypes.bfloat16

N_CORES = 8
D = 1024
S = 512
EPS = 1e-6


def split_sync_waits(nc, max_waits=1):
    """The pinned walrus accepts at most one sync wait per instruction;
    hoist excess waits onto same-engine NoOps inserted before the
    offending instruction (same engine => identical semantics)."""
    n = 0
    for bb in nc.main_func.blocks:
        out = []
        for inst in bb.instructions:
            si = inst.sync_info
            if si is not None and si.on_wait and len(si.on_wait) > max_waits:
                waits = list(si.on_wait)
                spill, keep = waits[:-max_waits], waits[-max_waits:]
                for j in range(0, len(spill), max_waits):
                    nop = mybir.InstNoOp(
                        name=f"{inst.name}_wsp{j}",
                        engine=inst.engine,
                        ins=[],
                        outs=[],
                        bass_nofuse=True,
                        sync_info=mybir.SyncInfo(
                            on_wait=spill[j : j + max_waits], on_update=[]
                        ),
                    )
                    nc.register_instruction(nop)
                    out.append(nop)
                    n += 1
                si.on_wait[:] = keep
            out.append(inst)
        bb.instructions[:] = out
    return n


def build_nc(T=2048, use_collective=True):
    NT = T // 128  # t-chunks (phase 1)
    NQ = T // 512  # q-chunks (phase 2)
    ND = D // 128  # contraction chunks over d
    NS = S // 128  # s-tiles
    NF = D // 128  # f-tiles (gate dim)

    nc = bass.Bass("TRN2", target_bir_lowering=False, debug=False,
                   num_devices=N_CORES)

    # ---- I/O ------------------------------------------------------------
    # all streaming tensors are host-prepacked so that every DMA source
    # slice is a [128, W] block with fully contiguous rows
    qTp = nc.dram_tensor("qTp", [NQ * 128, ND * 512], BF16,
                         kind="ExternalInput")
    kTp = nc.dram_tensor("kTp", [NT * 128, ND * 128], BF16,
                         kind="ExternalInput")
    vTp = nc.dram_tensor("vTp", [NT * 128, ND * 128], BF16,
                         kind="ExternalInput")
    wgp = nc.dram_tensor("wgp", [128, ND * D], BF16, kind="ExternalInput")
    wvp = nc.dram_tensor("wvp", [128, ND * D], BF16, kind="ExternalInput")
    wqp = nc.dram_tensor("wqp", [128, ND * S], BF16, kind="ExternalInput")
    wkp = nc.dram_tensor("wkp", [128, ND * S], BF16, kind="ExternalInput")
    wop = nc.dram_tensor("wop", [128, ND * D], BF16, kind="ExternalInput")
    # combined aux tensors: one descriptor each (dma_start costs ~610ns
    # of serial Sync-engine time, so descriptor count matters)
    # auxf layout: bk 0:512 | bo 512:1536 | km 1536:+NT | qm | bg | bq
    AUXF = S + D + NT + NT + NF + NS
    auxf = nc.dram_tensor("auxf", [128, AUXF], F32, kind="ExternalInput")
    # auxb layout: bv 0:1024 | km2 1024:1024+2*NT
    AUXB = D + 2 * NT
    auxb = nc.dram_tensor("auxb", [128, AUXB], BF16, kind="ExternalInput")
    out = nc.dram_tensor("out", [T, D], F32, kind="ExternalOutput")

    with tile.TileContext(nc) as tc:
        with tc.tile_pool(name="const", bufs=1) as cp:
            wk_sb = cp.tile([128, ND, S], BF16)
            auxf_sb = cp.tile([128, AUXF], F32)
            auxb_sb = cp.tile([128, AUXB], BF16)
            bk_sb = auxf_sb[:, 0:S]
            bo_bc = auxf_sb[:, S:S + D]
            km_sb = auxf_sb[:, S + D:S + D + NT]
            qm_sb = auxf_sb[:, S + D + NT:S + D + 2 * NT]
            bg_sb = auxf_sb[:, S + D + 2 * NT:S + D + 2 * NT + NF]
            bq_sb = auxf_sb[:, S + D + 2 * NT + NF:S + D + 2 * NT + NF + NS]
            bv_bc = auxb_sb[:, 0:D]
            km2_sb = auxb_sb[:, D:D + 2 * NT]
            wq_sb = cp.tile([128, ND, S], BF16)
            wg_sb = cp.tile([128, ND, D], BF16)
            wo_sb = cp.tile([128, ND, D], BF16)
            # kv result (post collective), lives through phase 2
            kv_sb = cp.tile([128, NS, D], BF16)
            ks_sb = cp.tile([128, 2 * NS], BF16)

            with tc.tile_pool(name="dram", bufs=1, space="DRAM") as dram, \
                 tc.tile_pool(name="pf", bufs=1) as pf:
                b0_in = dram.tile([S, S], BF16)
                b0_out = dram.tile([S, S], BF16)
                b1_in = dram.tile([S, S + 1], BF16)
                b1_out = dram.tile([S, S + 1], BF16)

                def load_qc(qch):
                    qc = pf.tile([128, ND, 512], BF16, name="qc",
                                 tag="qc", bufs=2)
                    nc.sync.dma_start(
                        qc[:], qTp.ap()[qch * 128:(qch + 1) * 128, :])
                    return qc
                qc_pre = {}

                # ================= phase 1: k features, v proj, kv =======
                with tc.tile_pool(name="p1", bufs=1) as p1, \
                     tc.tile_pool(name="ps1", bufs=1, space="PSUM") as ps1:
                    # startup-critical descriptors first: MM #1 needs
                    # kc[0] + wk; the vb matmuls need vc[0] + wv
                    def load_kvc(t):
                        kc = p1.tile([128, ND, 128], BF16, name="kc",
                                     tag="kc", bufs=3)
                        nc.sync.dma_start(kc[:], kTp.ap()[t * 128:(t + 1) * 128, :])
                        vc = p1.tile([128, ND, 128], BF16, name="vc",
                                     tag="vc", bufs=3)
                        nc.sync.dma_start(vc[:], vTp.ap()[t * 128:(t + 1) * 128, :])
                        return kc, vc

                    kc0vc0 = load_kvc(0)
                    nc.sync.dma_start(wk_sb[:], wkp.ap())
                    # wv arrives in two halves so the first vb matmuls
                    # only wait on 1MB; aux lands between the halves
                    wv_sb = p1.tile([128, 2, ND, S], BF16)
                    nc.sync.dma_start(wv_sb[:, 0], wvp.ap()[:, 0:ND * S])
                    nc.sync.dma_start(auxf_sb[:], auxf.ap())
                    nc.sync.dma_start(auxb_sb[:], auxb.ap())
                    nc.sync.dma_start(wv_sb[:, 1], wvp.ap()[:, ND * S:])

                    k_nat = p1.tile([128, NT, S], BF16)   # relu^2 key feats
                    v_e1 = p1.tile([128, NT, S], BF16)    # v proj, cols 512:

                    kv0 = [ps1.tile([128, S], F32, name=f"kv0_{s}", tag="kv0",
                                    bufs=NS) for s in range(NS)]

                    for t in range(NT):
                        kc, vc = kc0vc0 if t == 0 else load_kvc(t)
                        if t == 2:
                            qc_pre[0] = load_qc(0)
                        if t == 4:
                            nc.sync.dma_start(wq_sb[:], wqp.ap())
                        if t == 3:
                            nc.sync.dma_start(wg_sb[:], wgp.ap())
                        if t == 5:
                            nc.sync.dma_start(wo_sb[:], wop.ap())

                        # k features: relu(K Wk^T + bk)^2 -> k_nat[:,t,:]
                        kb = ps1.tile([128, S], F32, name="kb", tag="kb", bufs=2)
                        for c in range(ND):
                            nc.tensor.matmul(kb[:], kc[:, c, :], wk_sb[:, c, :],
                                             start=(c == 0), stop=(c == ND - 1))
                        kbb = p1.tile([128, S], F32, name="kbb",
                                      tag="kbb", bufs=2)
                        nc.vector.tensor_tensor(kbb[:], kb[:], bk_sb[:], ALU.add)
                        nc.vector.scalar_tensor_tensor(
                            k_nat[:, t, :], kbb[:], 0.0, kbb[:],
                            op0=ALU.max, op1=ALU.mult)

                        # v projection: V Wv^T (bias folded out; km scale
                        # applied here since km^2 == km) -> [128, 1024]
                        vb = ps1.tile([128, D], F32, name="vb", tag="vb", bufs=1)
                        for half in range(2):
                            for c in range(ND):
                                nc.tensor.matmul(
                                    vb[:, half * S:(half + 1) * S],
                                    vc[:, c, :],
                                    wv_sb[:, half, c, :],
                                    start=(c == 0), stop=(c == ND - 1))
                        v0 = p1.tile([128, S], BF16, name="v0", tag="v0", bufs=2)
                        nc.scalar.activation(v0[:], vb[:, 0:S], AF.Copy,
                                             scale=km_sb[:, t:t + 1])
                        nc.scalar.activation(v_e1[:, t, :], vb[:, S:2 * S],
                                             AF.Copy, scale=km_sb[:, t:t + 1])

                        # kv e-half 0 accumulates across the whole t loop
                        for s in range(NS):
                            nc.tensor.matmul(
                                kv0[s][:], k_nat[:, t, s * 128:(s + 1) * 128],
                                v0[:], start=(t == 0), stop=(t == NT - 1))

                    for s in range(NS):
                        kvst = p1.tile([128, S], BF16, name="kvst",
                                       tag="kvst", bufs=4)
                        # split the drains across scalar+vector so the
                        # PSUM banks free in ~half the serial time
                        if s % 2 == 0:
                            nc.scalar.activation(kvst[:], kv0[s][:], AF.Copy)
                        else:
                            nc.vector.tensor_copy(kvst[:], kv0[s][:])
                        nc.sync.dma_start(
                            b0_in[s * 128:(s + 1) * 128, :], kvst[:])

                if use_collective:
                    nc.gpsimd.collective_compute(
                        "AllReduce", ALU.add,
                        replica_groups=[[0, 1], [2, 3], [4, 5], [6, 7]],
                        ins=[b0_in.opt()], outs=[b0_out.opt()])
                    kv0_src = b0_out
                else:
                    kv0_src = b0_in

                # kv e-half 1 + k_sum (separate PSUM pool after ps1 frees)
                with tc.tile_pool(name="p1b", bufs=1) as p1b, \
                     tc.tile_pool(name="ps1b", bufs=1, space="PSUM") as ps1b:
                    for s in range(NS):
                        kv1 = ps1b.tile([128, S], F32, name=f"kv1_{s}",
                                        tag="kv1", bufs=2)
                        for t in range(NT):
                            nc.tensor.matmul(
                                kv1[:], k_nat[:, t, s * 128:(s + 1) * 128],
                                v_e1[:, t, :], start=(t == 0), stop=(t == NT - 1))
                        ks = ps1b.tile([128, 2], F32, name=f"ks_{s}",
                                       tag="ks", bufs=2)
                        for t in range(NT):
                            nc.tensor.matmul(
                                ks[:], k_nat[:, t, s * 128:(s + 1) * 128],
                                km2_sb[:, 2 * t:2 * t + 2],
                                start=(t == 0), stop=(t == NT - 1))
                        # single staging tile + single DMA per s-chunk
                        kvst1 = p1b.tile([128, S + 1], BF16, name="kvst1",
                                         tag="kvst1", bufs=2)
                        nc.scalar.activation(kvst1[:, 0:S], kv1[:], AF.Copy)
                        nc.vector.tensor_copy(kvst1[:, S:S + 1], ks[:, 0:1])
                        nc.sync.dma_start(
                            b1_in[s * 128:(s + 1) * 128, :], kvst1[:])

                if use_collective:
                    nc.gpsimd.collective_compute(
                        "AllReduce", ALU.add,
                        replica_groups=[[0, 1], [2, 3], [4, 5], [6, 7]],
                        ins=[b1_in.opt()], outs=[b1_out.opt()])
                    kv1_src = b1_out
                else:
                    kv1_src = b1_in

                with tc.tile_pool(name="p2", bufs=1) as p2, \
                     tc.tile_pool(name="ps2", bufs=1, space="PSUM") as ps2:
                    q_sb = p2.tile([128, NS, T], BF16)
                    u_sb = p2.tile([128, NF, T], BF16)

                    # ---- pass A: q features + u gate (no kv dependency) --
                    for qch in range(NQ):
                        t0 = qch * 512
                        qc = qc_pre.pop(qch, None)
                        if qc is None:
                            qc = load_qc(qch)
                        for s in range(NS):
                            qf = ps2.tile([128, 512], F32, name="qf",
                                          tag="mm", bufs=3)
                            for c in range(ND):
                                nc.tensor.matmul(
                                    qf[:], wq_sb[:, c, s * 128:(s + 1) * 128],
                                    qc[:, c, :], start=(c == 0), stop=(c == ND - 1))
                            qrelu = p2.tile([128, 512], F32, name="qrelu",
                                            tag="qrelu", bufs=2)
                            nc.scalar.activation(qrelu[:], qf[:], AF.Relu,
                                                 bias=bq_sb[:, s:s + 1])
                            nc.vector.tensor_mul(q_sb[:, s, t0:t0 + 512],
                                                 qrelu[:], qrelu[:])
                        for f in range(NF):
                            uf = ps2.tile([128, 512], F32, name="uf",
                                          tag="mm", bufs=3)
                            for c in range(ND):
                                nc.tensor.matmul(
                                    uf[:], wg_sb[:, c, f * 128:(f + 1) * 128],
                                    qc[:, c, :], start=(c == 0), stop=(c == ND - 1))
                            nc.scalar.activation(u_sb[:, f, t0:t0 + 512], uf[:],
                                                 AF.Silu, bias=bg_sb[:, f:f + 1])

                    # ---- unpack kv + k_sum (after pass A in engine order) -
                    # kv := allreduced(kv_nobias) + bv x k_sum  (rank-1 fix
                    # restores the v-projection bias)
                    ks_f = p2.tile([128, NS], BF16)
                    nc.sync.dma_start(
                        ks_f[:],
                        kv1_src[:, S:S + 1].rearrange("(c p) o -> p (c o)",
                                                      p=128))
                    for c in range(NS):
                        kv_f = p2.tile([128, D], BF16, name="kv_f",
                                       tag="kv_f", bufs=2)
                        nc.sync.dma_start(
                            kv_f[:, 0:S], kv0_src[c * 128:(c + 1) * 128, :])
                        nc.sync.dma_start(
                            kv_f[:, S:D], kv1_src[c * 128:(c + 1) * 128, 0:S])
                        nc.vector.scalar_tensor_tensor(
                            kv_sb[:, c, :], bv_bc[:], ks_f[:, c:c + 1], kv_f[:],
                            op0=ALU.mult, op1=ALU.add)
                    for c in range(NS):
                        for j in range(2):
                            nc.vector.tensor_copy(
                                ks_sb[:, 2 * c + j:2 * c + j + 1],
                                ks_f[:, c:c + 1])

                    # ---- pass B: qkv, z, gated output projection ---------
                    for qch in range(NQ):
                        t0 = qch * 512
                        for f in range(NF):
                            qk = ps2.tile([128, 512], F32, name="qk",
                                          tag="mm", bufs=3)
                            for c in range(NS):
                                nc.tensor.matmul(
                                    qk[:], kv_sb[:, c, f * 128:(f + 1) * 128],
                                    q_sb[:, c, t0:t0 + 512],
                                    start=(c == 0), stop=(c == NS - 1))
                            nc.vector.tensor_mul(u_sb[:, f, t0:t0 + 512], qk[:],
                                                 u_sb[:, f, t0:t0 + 512])
                        for tt in range(4):
                            ti = qch * 4 + tt
                            zp = ps2.tile([128, 2], F32, name="zp",
                                          tag="z", bufs=2)
                            for c in range(NS):
                                nc.tensor.matmul(
                                    zp[:],
                                    q_sb[:, c, ti * 128:(ti + 1) * 128],
                                    ks_sb[:, 2 * c:2 * c + 2], start=(c == 0),
                                    stop=(c == NS - 1))
                            z_sb = p2.tile([128, 1], F32, name="z_sb",
                                           tag="z_sb", bufs=2)
                            nc.vector.tensor_scalar_add(z_sb[:], zp[:, 0:1], EPS)
                            zi = p2.tile([128, 1], F32, name="zi",
                                         tag="zi", bufs=2)
                            nc.vector.reciprocal(zi[:], z_sb[:])
                            nc.vector.tensor_mul(zi[:], zi[:], qm_sb[:, ti:ti + 1])

                            o_sb = p2.tile([128, D], F32, name="o_sb",
                                           tag="o_sb", bufs=2)
                            for half in range(2):
                                op = ps2.tile([128, 512], F32, name="op",
                                              tag="out", bufs=2)
                                for f in range(NF):
                                    nc.tensor.matmul(
                                        op[:],
                                        u_sb[:, f, ti * 128:(ti + 1) * 128],
                                        wo_sb[:, f, half * S:(half + 1) * S],
                                        start=(f == 0), stop=(f == NF - 1))
                                nc.scalar.activation(
                                    o_sb[:, half * S:(half + 1) * S], op[:],
                                    AF.Copy, scale=zi[:])
                                nc.vector.scalar_tensor_tensor(
                                    o_sb[:, half * S:(half + 1) * S],
                                    bo_bc[:, half * S:(half + 1) * S],
                                    qm_sb[:, ti:ti + 1],
                                    o_sb[:, half * S:(half + 1) * S],
                                    op0=ALU.mult, op1=ALU.add)
                            nc.sync.dma_start(
                                out.ap()[ti * 128:(ti + 1) * 128, :], o_sb[:])

    split_sync_waits(nc)
    return nc


_NC_CACHE = {}


def _get_nc(T, use_collective=True):
    key = (T, use_collective)
    if key not in _NC_CACHE:
        _NC_CACHE[key] = build_nc(T, use_collective)
    return _NC_CACHE[key]


def make_in_maps(queries, keys, values, query_mask, key_mask,
                 Wg, bg, Wv, bv, Wq, bq, Wk, bk, Wo, bo):
    B, T_full, _ = queries.shape
    Th = T_full // 2
    NT = Th // 128
    f32 = np.float32
    qTb = np.ascontiguousarray(queries.transpose(0, 2, 1)).astype(NPBF)
    kTb = np.ascontiguousarray(keys.transpose(0, 2, 1)).astype(NPBF)
    vTb = np.ascontiguousarray(values.transpose(0, 2, 1)).astype(NPBF)

    def packw(W):
        # [D, F] -> [128, ND*F]: row p = concat_c W[c*128+p, :]
        wt = np.asarray(W, f32).T.astype(NPBF)
        return np.ascontiguousarray(
            wt.reshape(-1, 128, wt.shape[1]).transpose(1, 0, 2)
            .reshape(128, -1))

    def packt(A, chunk):
        # [D, Th] -> [(Th//chunk)*128, ND*chunk]: block t row p =
        # concat_c A[c*128+p, t*chunk:(t+1)*chunk]
        nd = A.shape[0] // 128
        nt = A.shape[1] // chunk
        return np.ascontiguousarray(
            A.reshape(nd, 128, nt, chunk).transpose(2, 1, 0, 3)
            .reshape(nt * 128, nd * chunk))

    wvt = np.asarray(Wv, f32).T.astype(NPBF)  # [D, D]
    wv_halves = [
        wvt[:, h * S:(h + 1) * S].reshape(-1, 128, S).transpose(1, 0, 2)
        .reshape(128, -1)
        for h in range(2)
    ]
    shared = {
        "wgp": packw(Wg),
        "wvp": np.ascontiguousarray(np.concatenate(wv_halves, axis=1)),
        "wqp": packw(Wq),
        "wkp": packw(Wk),
        "wop": packw(Wo),
    }
    bk_bc = np.tile(np.asarray(bk, f32).reshape(1, S), (128, 1))
    bo_bc = np.tile(np.asarray(bo, f32).reshape(1, D), (128, 1))
    bg_p = np.asarray(bg, f32).reshape(-1, 128).T
    bq_p = np.asarray(bq, f32).reshape(-1, 128).T
    bv_bc = np.tile(np.asarray(bv, f32).reshape(1, D), (128, 1)).astype(NPBF)
    in_maps = []
    for c in range(N_CORES):
        b, h = divmod(c, 2)
        sl = slice(h * Th, (h + 1) * Th)
        m = dict(shared)
        m["qTp"] = packt(qTb[b][:, sl], 512)
        m["kTp"] = packt(kTb[b][:, sl], 128)
        m["vTp"] = packt(vTb[b][:, sl], 128)
        km = np.asarray(key_mask[b, sl], f32).reshape(NT, 128).T
        qm = np.asarray(query_mask[b, sl], f32).reshape(NT, 128).T
        km2 = np.repeat(km, 2, axis=1)
        # auxf: bk | bo | km | qm | bg | bq   auxb: bv | km2
        m["auxf"] = np.ascontiguousarray(
            np.concatenate([bk_bc, bo_bc, km, qm, bg_p, bq_p], axis=1))
        m["auxb"] = np.ascontiguousarray(
            np.concatenate([bv_bc, km2.astype(NPBF)], axis=1))
        in_maps.append(m)
    return in_maps


def kernel(queries, keys, values, query_mask, key_mask,
           Wg, bg, Wv, bv, Wq, bq, Wk, bk, Wo, bo, _trace=False):
    B, T_full, _ = queries.shape
    Th = T_full // 2
    nc = _get_nc(Th)
    in_maps = make_in_maps(queries, keys, values, query_mask, key_mask,
                           Wg, bg, Wv, bv, Wq, bq, Wk, bk, Wo, bo)
    res = run_bass_kernel_spmd(nc, in_maps, core_ids=list(range(N_CORES)),
                               trace=_trace)
    out = np.empty((B, T_full, D), np.float32)
    for c in range(N_CORES):
        b, h = divmod(c, 2)
        out[b, h * Th:(h + 1) * Th] = res.results[c]["out"]
    if _trace:
        kernel._last_res = res
    return out


# revision 17
# speedup vs baseline: 1.0349x; 1.0349x over previous
"""GAU (gated attention unit, relu^2 linear attention) Trainium2 kernel.

Sharding: 8 cores = batch (4) x T-half (2).  Each core handles 2048 rows
of both the query and key/value streams of one batch.  The kv/k_sum
reduction over keys is completed with two 2-rank AllReduces (bf16
payload), each issued as soon as its half of the data is ready so both
hide under the q-feature/gate pass.

Matmuls run in bf16 with fp32 PSUM accumulation; inputs are cast to
bf16 on the host.  Bias adds ride vector/scalar ops instead of PE
matmuls: bk via a vector add, bv via a rank-1 k_sum x bv fix-up fused
into the kv unpack, and the key mask is folded into the v-projection
copy (km^2 == km for 0/1 masks) and the k_sum ones-vector.
"""
import sys

sys.path.insert(0, "/opt/trn_rl_repo")

import ml_dtypes
import numpy as np
import concourse.bass as bass
import concourse.mybir as mybir
import concourse.tile as tile
from concourse.bass_utils import run_bass_kernel_spmd

AF = mybir.ActivationFunctionType
ALU = mybir.AluOpType
F32 = mybir.dt.float32
BF16 = mybir.dt.bfloat16
NPBF = ml_dtypes.bfloat16

N_CORES = 8
D = 1024
S = 512
EPS = 1e-6


def split_sync_waits(nc, max_waits=1):
    """The pinned walrus accepts at most one sync wait per instruction;
    hoist excess waits onto same-engine NoOps inserted before the
    offending instruction (same engine => identical semantics)."""
    n = 0
    for bb in nc.main_func.blocks:
        out = []
        for inst in bb.instructions:
            si = inst.sync_info
            if si is not None and si.on_wait and len(si.on_wait) > max_waits:
                waits = list(si.on_wait)
                spill, keep = waits[:-max_waits], waits[-max_waits:]
                for j in range(0, len(spill), max_waits):
                    nop = mybir.InstNoOp(
                        name=f"{inst.name}_wsp{j}",
                        engine=inst.engine,
                        ins=[],
                        outs=[],
                        bass_nofuse=True,
                        sync_info=mybir.SyncInfo(
                            on_wait=spill[j : j + max_waits], on_update=[]
                        ),
                    )
                    nc.register_instruction(nop)
                    out.append(nop)
                    n += 1
                si.on_wait[:] = keep
            out.append(inst)
        bb.instructions[:] = out
    return n


def build_nc(T=2048, use_collective=True):
    NT = T // 128  # t-chunks (phase 1)
    NQ = T // 512  # q-chunks (phase 2)
    ND = D // 128  # contraction chunks over d
    NS = S // 128  # s-tiles
    NF = D // 128  # f-tiles (gate dim)

    nc = bass.Bass("TRN2", target_bir_lowering=False, debug=False,
                   num_devices=N_CORES)

    # ---- I/O ------------------------------------------------------------
    # all streaming tensors are host-prepacked so that every DMA source
    # slice is a [128, W] block with fully contiguous rows
    qTp = nc.dram_tensor("qTp", [NQ * 128, ND * 512], BF16,
                         kind="ExternalInput")
    kTp = nc.dram_tensor("kTp", [NT * 128, ND * 128], BF16,
                         kind="ExternalInput")
    vTp = nc.dram_tensor("vTp", [NT * 128, ND * 128], BF16,
                         kind="ExternalInput")
    wgp = nc.dram_tensor("wgp", [128, ND * D], BF16, kind="ExternalInput")
    wvp = nc.dram_tensor("wvp", [128, ND * D], BF16, kind="ExternalInput")
    wqp = nc.dram_tensor("wqp", [128, ND * S], BF16, kind="ExternalInput")
    wkp = nc.dram_tensor("wkp", [128, ND * S], BF16, kind="ExternalInput")
    wop = nc.dram_tensor("wop", [128, ND * D], BF16, kind="ExternalInput")
    # combined aux tensors: one descriptor each (dma_start costs ~610ns
    # of serial Sync-engine time, so descriptor count matters)
    # auxf layout: bk 0:512 | bo 512:1536 | km 1536:+NT | qm | bg | bq
    AUXF = S + D + NT + NT + NF + NS
    auxf = nc.dram_tensor("auxf", [128, AUXF], F32, kind="ExternalInput")
    # auxb layout: bv 0:1024 | km2 1024:1024+2*NT
    AUXB = D + 2 * NT
    auxb = nc.dram_tensor("auxb", [128, AUXB], BF16, kind="ExternalInput")
    out = nc.dram_tensor("out", [T, D], F32, kind="ExternalOutput")

    with tile.TileContext(nc) as tc:
        with tc.tile_pool(name="const", bufs=1) as cp:
            wk_sb = cp.tile([128, ND, S], BF16)
            auxf_sb = cp.tile([128, AUXF], F32)
            auxb_sb = cp.tile([128, AUXB], BF16)
            bk_sb = auxf_sb[:, 0:S]
            bo_bc = auxf_sb[:, S:S + D]
            km_sb = auxf_sb[:, S + D:S + D + NT]
            qm_sb = auxf_sb[:, S + D + NT:S + D + 2 * NT]
            bg_sb = auxf_sb[:, S + D + 2 * NT:S + D + 2 * NT + NF]
            bq_sb = auxf_sb[:, S + D + 2 * NT + NF:S + D + 2 * NT + NF + NS]
            bv_bc = auxb_sb[:, 0:D]
            km2_sb = auxb_sb[:, D:D + 2 * NT]
            wq_sb = cp.tile([128, ND, S], BF16)
            wg_sb = cp.tile([128, ND, D], BF16)
            wo_sb = cp.tile([128, ND, D], BF16)
            # kv result (post collective), lives through phase 2
            kv_sb = cp.tile([128, NS, D], BF16)
            ks_sb = cp.tile([128, 2 * NS], BF16)

            with tc.tile_pool(name="dram", bufs=1, space="DRAM") as dram, \
                 tc.tile_pool(name="pf", bufs=1) as pf:
                b0_in = dram.tile([S, S], BF16)
                b0_out = dram.tile([S, S], BF16)
                b1_in = dram.tile([S, S + 1], BF16)
                b1_out = dram.tile([S, S + 1], BF16)

                def load_qc(qch):
                    qc = pf.tile([128, ND, 512], BF16, name="qc",
                                 tag="qc", bufs=2)
                    nc.sync.dma_start(
                        qc[:], qTp.ap()[qch * 128:(qch + 1) * 128, :])
                    return qc
                qc_pre = {}

                # ================= phase 1: k features, v proj, kv =======
                with tc.tile_pool(name="p1", bufs=1) as p1, \
                     tc.tile_pool(name="ps1", bufs=1, space="PSUM") as ps1:
                    # startup-critical descriptors first: MM #1 needs
                    # kc[0] + wk; the vb matmuls need vc[0] + wv
                    def load_kvc(t):
                        kc = p1.tile([128, ND, 128], BF16, name="kc",
                                     tag="kc", bufs=3)
                        nc.sync.dma_start(kc[:], kTp.ap()[t * 128:(t + 1) * 128, :])
                        vc = p1.tile([128, ND, 128], BF16, name="vc",
                                     tag="vc", bufs=3)
                        nc.sync.dma_start(vc[:], vTp.ap()[t * 128:(t + 1) * 128, :])
                        return kc, vc

                    kvc0 = load_kvc(0)
                    nc.sync.dma_start(wk_sb[:], wkp.ap())
                    wv_sb = p1.tile([128, ND, D], BF16)
                    nc.sync.dma_start(wv_sb[:], wvp.ap())
                    nc.sync.dma_start(auxf_sb[:], auxf.ap())
                    nc.sync.dma_start(auxb_sb[:], auxb.ap())

                    k_nat = p1.tile([128, NT, S], BF16)   # relu^2 key feats
                    v_e1 = p1.tile([128, NT, S], BF16)    # v proj, cols 512:

                    kv0 = [ps1.tile([128, S], F32, name=f"kv0_{s}", tag="kv0",
                                    bufs=NS) for s in range(NS)]

                    for t in range(NT):
                        kc, vc = kvc0 if t == 0 else load_kvc(t)
                        if t == 1:
                            qc_pre[0] = load_qc(0)
                            nc.sync.dma_start(wq_sb[:], wqp.ap())
                        if t == 3:
                            nc.sync.dma_start(wg_sb[:], wgp.ap())
                        if t == 5:
                            nc.sync.dma_start(wo_sb[:], wop.ap())

                        # k features: relu(K Wk^T + bk)^2 -> k_nat[:,t,:]
                        kb = ps1.tile([128, S], F32, name="kb", tag="kb", bufs=2)
                        for c in range(ND):
                            nc.tensor.matmul(kb[:], kc[:, c, :], wk_sb[:, c, :],
                                             start=(c == 0), stop=(c == ND - 1))
                        kbb = p1.tile([128, S], F32, name="kbb",
                                      tag="kbb", bufs=2)
                        nc.vector.tensor_tensor(kbb[:], kb[:], bk_sb[:], ALU.add)
                        nc.vector.scalar_tensor_tensor(
                            k_nat[:, t, :], kbb[:], 0.0, kbb[:],
                            op0=ALU.max, op1=ALU.mult)

                        # v projection: V Wv^T (bias folded out; km scale
                        # applied here since km^2 == km) -> [128, 1024]
                        vb = ps1.tile([128, D], F32, name="vb", tag="vb", bufs=1)
                        for half in range(2):
                            for c in range(ND):
                                nc.tensor.matmul(
                                    vb[:, half * S:(half + 1) * S],
                                    vc[:, c, :],
                                    wv_sb[:, c, half * S:(half + 1) * S],
                                    start=(c == 0), stop=(c == ND - 1))
                        v0 = p1.tile([128, S], BF16, name="v0", tag="v0", bufs=2)
                        nc.scalar.activation(v0[:], vb[:, 0:S], AF.Copy,
                                             scale=km_sb[:, t:t + 1])
                        nc.scalar.activation(v_e1[:, t, :], vb[:, S:2 * S],
                                             AF.Copy, scale=km_sb[:, t:t + 1])

                        # kv e-half 0 accumulates across the whole t loop
                        for s in range(NS):
                            nc.tensor.matmul(
                                kv0[s][:], k_nat[:, t, s * 128:(s + 1) * 128],
                                v0[:], start=(t == 0), stop=(t == NT - 1))

                    for s in range(NS):
                        kvst = p1.tile([128, S], BF16, name="kvst",
                                       tag="kvst", bufs=4)
                        # split the drains across scalar+vector so the
                        # PSUM banks free in ~half the serial time
                        if s % 2 == 0:
                            nc.scalar.activation(kvst[:], kv0[s][:], AF.Copy)
                        else:
                            nc.vector.tensor_copy(kvst[:], kv0[s][:])
                        nc.sync.dma_start(
                            b0_in[s * 128:(s + 1) * 128, :], kvst[:])

                if use_collective:
                    nc.gpsimd.collective_compute(
                        "AllReduce", ALU.add,
                        replica_groups=[[0, 1], [2, 3], [4, 5], [6, 7]],
                        ins=[b0_in.opt()], outs=[b0_out.opt()])
                    kv0_src = b0_out
                else:
                    kv0_src = b0_in

                # kv e-half 1 + k_sum (separate PSUM pool after ps1 frees)
                with tc.tile_pool(name="p1b", bufs=1) as p1b, \
                     tc.tile_pool(name="ps1b", bufs=1, space="PSUM") as ps1b:
                    for s in range(NS):
                        kv1 = ps1b.tile([128, S], F32, name=f"kv1_{s}",
                                        tag="kv1", bufs=2)
                        for t in range(NT):
                            nc.tensor.matmul(
                                kv1[:], k_nat[:, t, s * 128:(s + 1) * 128],
                                v_e1[:, t, :], start=(t == 0), stop=(t == NT - 1))
                        ks = ps1b.tile([128, 2], F32, name=f"ks_{s}",
                                       tag="ks", bufs=2)
                        for t in range(NT):
                            nc.tensor.matmul(
                                ks[:], k_nat[:, t, s * 128:(s + 1) * 128],
                                km2_sb[:, 2 * t:2 * t + 2],
                                start=(t == 0), stop=(t == NT - 1))
                        # single staging tile + single DMA per s-chunk
                        kvst1 = p1b.tile([128, S + 1], BF16, name="kvst1",
                                         tag="kvst1", bufs=2)
                        nc.scalar.activation(kvst1[:, 0:S], kv1[:], AF.Copy)
                        nc.vector.tensor_copy(kvst1[:, S:S + 1], ks[:, 0:1])
                        nc.sync.dma_start(
                            b1_in[s * 128:(s + 1) * 128, :], kvst1[:])

                if use_collective:
                    nc.gpsimd.collective_compute(
                        "AllReduce", ALU.add,
                        replica_groups=[[0, 1], [2, 3], [4, 5], [6, 7]],
                        ins=[b1_in.opt()], outs=[b1_out.opt()])
                    kv1_src = b1_out
                else:
                    kv1_src = b1_in

                with tc.tile_pool(name="p2", bufs=1) as p2, \
                     tc.tile_pool(name="ps2", bufs=1, space="PSUM") as ps2:
                    q_sb = p2.tile([128, NS, T], BF16)
                    u_sb = p2.tile([128, NF, T], BF16)

                    # ---- pass A: q features + u gate (no kv dependency) --
                    for qch in range(NQ):
                        t0 = qch * 512
                        qc = qc_pre.pop(qch, None)
                        if qc is None:
                            qc = load_qc(qch)
                        for s in range(NS):
                            qf = ps2.tile([128, 512], F32, name="qf",
                                          tag="mm", bufs=3)
                            for c in range(ND):
                                nc.tensor.matmul(
                                    qf[:], wq_sb[:, c, s * 128:(s + 1) * 128],
                                    qc[:, c, :], start=(c == 0), stop=(c == ND - 1))
                            qrelu = p2.tile([128, 512], F32, name="qrelu",
                                            tag="qrelu", bufs=2)
                            nc.scalar.activation(qrelu[:], qf[:], AF.Relu,
                                                 bias=bq_sb[:, s:s + 1])
                            nc.vector.tensor_mul(q_sb[:, s, t0:t0 + 512],
                                                 qrelu[:], qrelu[:])
                        for f in range(NF):
                            uf = ps2.tile([128, 512], F32, name="uf",
                                          tag="mm", bufs=3)
                            for c in range(ND):
                                nc.tensor.matmul(
                                    uf[:], wg_sb[:, c, f * 128:(f + 1) * 128],
                                    qc[:, c, :], start=(c == 0), stop=(c == ND - 1))
                            nc.scalar.activation(u_sb[:, f, t0:t0 + 512], uf[:],
                                                 AF.Silu, bias=bg_sb[:, f:f + 1])

                    # ---- unpack kv + k_sum (after pass A in engine order) -
                    # kv := allreduced(kv_nobias) + bv x k_sum  (rank-1 fix
                    # restores the v-projection bias)
                    ks_f = p2.tile([128, NS], BF16)
                    nc.sync.dma_start(
                        ks_f[:],
                        kv1_src[:, S:S + 1].rearrange("(c p) o -> p (c o)",
                                                      p=128))
                    for c in range(NS):
                        kv_f = p2.tile([128, D], BF16, name="kv_f",
                                       tag="kv_f", bufs=2)
                        nc.sync.dma_start(
                            kv_f[:, 0:S], kv0_src[c * 128:(c + 1) * 128, :])
                        nc.sync.dma_start(
                            kv_f[:, S:D], kv1_src[c * 128:(c + 1) * 128, 0:S])
                        nc.vector.scalar_tensor_tensor(
                            kv_sb[:, c, :], bv_bc[:], ks_f[:, c:c + 1], kv_f[:],
                            op0=ALU.mult, op1=ALU.add)
                    for c in range(NS):
                        for j in range(2):
                            nc.vector.tensor_copy(
                                ks_sb[:, 2 * c + j:2 * c + j + 1],
                                ks_f[:, c:c + 1])

                    # ---- pass B: qkv, z, gated output projection ---------
                    for qch in range(NQ):
                        t0 = qch * 512
                        for f in range(NF):
                            qk = ps2.tile([128, 512], F32, name="qk",
                                          tag="mm", bufs=3)
                            for c in range(NS):
                                nc.tensor.matmul(
                                    qk[:], kv_sb[:, c, f * 128:(f + 1) * 128],
                                    q_sb[:, c, t0:t0 + 512],
                                    start=(c == 0), stop=(c == NS - 1))
                            nc.vector.tensor_mul(u_sb[:, f, t0:t0 + 512], qk[:],
                                                 u_sb[:, f, t0:t0 + 512])
                        for tt in range(4):
                            ti = qch * 4 + tt
                            zp = ps2.tile([128, 2], F32, name="zp",
                                          tag="z", bufs=2)
                            for c in range(NS):
                                nc.tensor.matmul(
                                    zp[:],
                                    q_sb[:, c, ti * 128:(ti + 1) * 128],
                                    ks_sb[:, 2 * c:2 * c + 2], start=(c == 0),
                                    stop=(c == NS - 1))
                            z_sb = p2.tile([128, 1], F32, name="z_sb",
                                           tag="z_sb", bufs=2)
                            nc.vector.tensor_scalar_add(z_sb[:], zp[:, 0:1], EPS)
                            zi = p2.tile([128, 1], F32, name="zi",
                                         tag="zi", bufs=2)
                            nc.vector.reciprocal(zi[:], z_sb[:])
                            nc.vector.tensor_mul(zi[:], zi[:], qm_sb[:, ti:ti + 1])

                            o_sb = p2.tile([128, D], F32, name="o_sb",
                                           tag="o_sb", bufs=2)
                            for half in range(2):
                                op = ps2.tile([128, 512], F32, name="op",
                                              tag="out", bufs=2)
                                for f in range(NF):
                                    nc.tensor.matmul(
                                        op[:],
                                        u_sb[:, f, ti * 128:(ti + 1) * 128],
                                        wo_sb[:, f, half * S:(half + 1) * S],
                                        start=(f == 0), stop=(f == NF - 1))
                                nc.scalar.activation(
                                    o_sb[:, half * S:(half + 1) * S], op[:],
                                    AF.Copy, scale=zi[:])
                                nc.vector.scalar_tensor_tensor(
                                    o_sb[:, half * S:(half + 1) * S],
                                    bo_bc[:, half * S:(half + 1) * S],
                                    qm_sb[:, ti:ti + 1],
                                    o_sb[:, half * S:(half + 1) * S],
                                    op0=ALU.mult, op1=ALU.add)
                            nc.sync.dma_start(
                                out.ap()[ti * 128:(ti + 1) * 128, :], o_sb[:])

    split_sync_waits(nc)
    return nc


_NC_CACHE = {}


def _get_nc(T, use_collective=True):
    key = (T, use_collective)
    if key not in _NC_CACHE:
        _NC_CACHE[key] = build_nc(T, use_collective)
    return _NC_CACHE[key]


def make_in_maps(queries, keys, values, query_mask, key_mask,
                 Wg, bg, Wv, bv, Wq, bq, Wk, bk, Wo, bo):
    B, T_full, _ = queries.shape
    Th = T_full // 2
    NT = Th // 128
    f32 = np.float32
    qTb = np.ascontiguousarray(queries.transpose(0, 2, 1)).astype(NPBF)
    kTb = np.ascontiguousarray(keys.transpose(0, 2, 1)).astype(NPBF)
    vTb = np.ascontiguousarray(values.transpose(0, 2, 1)).astype(NPBF)

    def packw(W):
        # [D, F] -> [128, ND*F]: row p = concat_c W[c*128+p, :]
        wt = np.asarray(W, f32).T.astype(NPBF)
        return np.ascontiguousarray(
            wt.reshape(-1, 128, wt.shape[1]).transpose(1, 0, 2)
            .reshape(128, -1))

    def packt(A, chunk):
        # [D, Th] -> [(Th//chunk)*128, ND*chunk]: block t row p =
        # concat_c A[c*128+p, t*chunk:(t+1)*chunk]
        nd = A.shape[0] // 128
        nt = A.shape[1] // chunk
        return np.ascontiguousarray(
            A.reshape(nd, 128, nt, chunk).transpose(2, 1, 0, 3)
            .reshape(nt * 128, nd * chunk))

    shared = {
        "wgp": packw(Wg),
        "wvp": packw(Wv),
        "wqp": packw(Wq),
        "wkp": packw(Wk),
        "wop": packw(Wo),
    }
    bk_bc = np.tile(np.asarray(bk, f32).reshape(1, S), (128, 1))
    bo_bc = np.tile(np.asarray(bo, f32).reshape(1, D), (128, 1))
    bg_p = np.asarray(bg, f32).reshape(-1, 128).T
    bq_p = np.asarray(bq, f32).reshape(-1, 128).T
    bv_bc = np.tile(np.asarray(bv, f32).reshape(1, D), (128, 1)).astype(NPBF)
    in_maps = []
    for c in range(N_CORES):
        b, h = divmod(c, 2)
        sl = slice(h * Th, (h + 1) * Th)
        m = dict(shared)
        m["qTp"] = packt(qTb[b][:, sl], 512)
        m["kTp"] = packt(kTb[b][:, sl], 128)
        m["vTp"] = packt(vTb[b][:, sl], 128)
        km = np.asarray(key_mask[b, sl], f32).reshape(NT, 128).T
        qm = np.asarray(query_mask[b, sl], f32).reshape(NT, 128).T
        km2 = np.repeat(km, 2, axis=1)
        # auxf: bk | bo | km | qm | bg | bq   auxb: bv | km2
        m["auxf"] = np.ascontiguousarray(
            np.concatenate([bk_bc, bo_bc, km, qm, bg_p, bq_p], axis=1))
        m["auxb"] = np.ascontiguousarray(
            np.concatenate([bv_bc, km2.astype(NPBF)], axis=1))
        in_maps.append(m)
    return in_maps


def kernel(queries, keys, values, query_mask, key_mask,
           Wg, bg, Wv, bv, Wq, bq, Wk, bk, Wo, bo, _trace=False):
    B, T_full, _ = queries.shape
    Th = T_full // 2
    nc = _get_nc(Th)
    in_maps = make_in_maps(queries, keys, values, query_mask, key_mask,
                           Wg, bg, Wv, bv, Wq, bq, Wk, bk, Wo, bo)
    res = run_bass_kernel_spmd(nc, in_maps, core_ids=list(range(N_CORES)),
                               trace=_trace)
    out = np.empty((B, T_full, D), np.float32)
    for c in range(N_CORES):
        b, h = divmod(c, 2)
        out[b, h * Th:(h + 1) * Th] = res.results[c]["out"]
    if _trace:
        kernel._last_res = res
    return out
